# revision 2
# baseline (speedup 1.0000x reference)
"""Trainium2 Bass kernel for AttentionBlock (B=8, C=256, L=2048), data-parallel
over batch across 8 NeuronCores.

Math (one batch per core, x: [C, L]):
    scores^T = x^T M x + (u.x) 1^T   with  M = Wq^T Wk,  u = Wk^T bq / sqrt(C)
    pT = exp(scores^T / sqrt(C) + ux)        [m, l], m on partitions
    denom = ones^T acc(pT)
    ctx = vT^T pT,  vT = x^T Wv^T
    out = ctx * (1/denom) + (x + bv)

All big matmuls run fp8e4m3 with DoubleRow (2 contraction rows per PE cell,
~1.5-2x bf16 throughput). Host scales M/Wv by 16 and u by 256 so fp8
operands sit in the normal range; the 16x on v cancels through the
denominator (its reduction matmul uses a constant-16 stationary) and the
256x on u is undone at the ux eviction.

exp is split across two engines: most [128,2048] chunks on ScalarE
(ACTIVATE Exp, fp8 out), a few on the DVE via a Schraudolph bit-trick -
one tensor_scalar computing int8( scores*a + b ) whose int8 bits ARE the
fp8e4m3 encoding of exp (validated offline: total rel err ~2.6e-3).
The per-chunk denominator accumulation (dacc, bf16) also rides the DVE.

Epilogue per 512-col slice: t = ctx*recip (DVE), out = (x + bv) + t in a
single scalar_tensor_tensor, then DMA; the residual x rides in fp32.
"""

import numpy as np
import ml_dtypes

import concourse.bass as bass
import concourse.tile as tile
from concourse import bacc, mybir
from concourse.bass_utils import run_bass_kernel_spmd

B, C, L = 8, 256, 2048
P = 128                 # partitions
NMC = L // P            # 16 m-chunks (key blocks)
NB = 512                # matmul moving free dim (one PSUM bank)
NLN = L // NB           # 4 col slices of 512
SCALE = float(C) ** -0.5
WARMUP_MMS = 6

LN2 = float(np.log(2.0))
EXP_A = 8.0 / (256.0 * LN2)          # scores_psum -> fp8 bits slope
EXP_C = 56.0 + 0.5                   # fp8e4m3 exponent-bias magic + tweak
UXB_A = 8.0 / LN2                    # ux -> bits bias slope

DVE_EXP_CHUNKS = (2, 5, 8, 11, 14)   # chunks whose exp runs on DVE

F32 = mybir.dt.float32
BF16 = mybir.dt.bfloat16
FP8 = mybir.dt.float8e4
I8 = mybir.dt.int8
DR = mybir.MatmulPerfMode.DoubleRow
MUL = mybir.AluOpType.mult
ADD = mybir.AluOpType.add

_COMPILED = None


def build_nc():
    nc = bacc.Bacc("TRN2", target_bir_lowering=False, debug=False, num_devices=8)

    x_d = nc.dram_tensor("x", [C, L], F32, kind="ExternalInput").ap()
    x8_d = nc.dram_tensor("x8", [C, L], FP8, kind="ExternalInput").ap()
    mt8_d = nc.dram_tensor("mt8", [C, C], FP8, kind="ExternalInput").ap()
    wvt8_d = nc.dram_tensor("wvt8", [C, C], FP8, kind="ExternalInput").ap()
    u8_d = nc.dram_tensor("u8", [C, 16], FP8, kind="ExternalInput").ap()
    bv_d = nc.dram_tensor("bv", [C, 1], F32, kind="ExternalInput").ap()
    out_d = nc.dram_tensor("out", [C, L], F32, kind="ExternalOutput").ap()
    uxs_d = nc.dram_tensor("uxs", [1, L], F32).ap()      # scratch bounce

    with tile.TileContext(nc) as tc:
        with (
            tc.tile_pool(name="const", bufs=1) as const,
            tc.tile_pool(name="data", bufs=1) as data,
            tc.tile_pool(name="evict", bufs=4) as evict,
        ):
            # ---- constants ----
            ones8 = const.tile([P, 2, NB], FP8)
            nc.gpsimd.memset(ones8[:], 1.0)
            cst16 = const.tile([P, P], BF16)
            nc.gpsimd.memset(cst16[:], 16.0)

            x8 = data.tile([P, 2, L], FP8, tag="x8", name="x8")
            mt8 = const.tile([P, 2, C], FP8, tag="mt8")
            wvt8 = const.tile([P, 2, C], FP8, tag="wvt8")
            u8 = const.tile([P, 2, 16], FP8, tag="u8")
            bv_sb = const.tile([P, 2, 1], F32, tag="bv")
            x_f = data.tile([P, 2, L], F32, tag="xf", name="xf")

            nc.sync.dma_start(
                out=x8[:, :, 0:1024],
                in_=x8_d.rearrange("(a p) l -> p a l", p=P)[:, :, 0:1024])
            nc.scalar.dma_start(
                out=x8[:, :, 1024:2048],
                in_=x8_d.rearrange("(a p) l -> p a l", p=P)[:, :, 1024:2048])
            nc.scalar.dma_start(out=mt8[:], in_=mt8_d.rearrange("(a p) c -> p a c", p=P))
            nc.scalar.dma_start(out=wvt8[:], in_=wvt8_d.rearrange("(a p) c -> p a c", p=P))
            nc.scalar.dma_start(out=u8[:], in_=u8_d.rearrange("(a p) o -> p a o", p=P))
            nc.scalar.dma_start(out=bv_sb[:], in_=bv_d.rearrange("(a p) o -> p a o", p=P))
            # residual fp32 x - only needed by the epilogue
            for cc in range(2):
                nc.sync.dma_start(out=x_f[:, cc, :],
                                  in_=x_d[cc * P:(cc + 1) * P, :])

            w8 = data.tile([P, 2, L], FP8, tag="w8", name="w8")
            vT8 = data.tile([P, NMC, C], FP8, tag="vT8")
            pT8 = data.tile([P, NMC, L], FP8, tag="pT8")
            dacc = data.tile([P, L], BF16, tag="dacc")
            recip = data.tile([P, L], F32, tag="recip")
            ux_row = data.tile([1, L], F32, tag="uxrow")
            ux_col = data.tile([P, NMC, 1], F32, tag="uxcol")
            uxb_col = data.tile([P, NMC, 1], F32, tag="uxbcol")
            junk = data.tile([P, 16], BF16, tag="junk")

            # warm the exp activation table set while DMAs land
            nc.scalar.activation(out=junk[:], in_=ones8[:, 0, 0:16],
                                 func=mybir.ActivationFunctionType.Exp)

            # ---- phase 1: projections ----
            with tc.tile_pool(name="psA", bufs=1, space=bass.MemorySpace.PSUM) as psA:
                warm = psA.tile([P, NB], F32, tag="warm", name="warm", bufs=1)
                for _ in range(WARMUP_MMS):
                    nc.tensor.matmul(warm[:], ones8[:, 0:2, 0:P],
                                     ones8[:, 0:2, 0:NB],
                                     start=True, stop=True, perf_mode=DR)

                def ux_chain(ln):
                    cols = slice(ln * NB, (ln + 1) * NB)
                    up = psA.tile([1, NB], F32, tag="up", name="up", bufs=1)
                    nc.tensor.matmul(up[0:1, :], u8[:, 0:2, 0:1],
                                     x8[:, 0:2, cols],
                                     start=True, stop=True, perf_mode=DR)
                    nc.vector.tensor_scalar_mul(ux_row[0:1, cols], up[:], 1.0 / 256.0)
                    nc.gpsimd.dma_start(out=uxs_d[0:1, cols], in_=ux_row[0:1, cols])
                    nc.gpsimd.dma_start(
                        out=ux_col[:, ln * 4:(ln + 1) * 4, :],
                        in_=uxs_d[0:1, cols].rearrange("o (a p) -> p a o", p=P))

                ux_chain(0)

                # w = (16M) x : DoubleRow contracts all 256 channels per mm
                for h in range(2):
                    for oc in range(2):
                        wp = psA.tile([P, 1024], F32, tag="big", name="wp", bufs=3)
                        for ln in range(2):
                            c0 = h * 1024 + ln * NB
                            nc.tensor.matmul(
                                wp[:, ln * NB:(ln + 1) * NB],
                                mt8[:, 0:2, oc * P:(oc + 1) * P],
                                x8[:, 0:2, c0:c0 + NB],
                                start=True, stop=True, perf_mode=DR)
                        nc.scalar.copy(out=w8[:, oc, h * 1024:(h + 1) * 1024],
                                       in_=wp[:])
                ux_chain(1)

                # vT[m, c] = sum_c' x[c', m] (16 WvT)[c', c]
                for qh in range(4):
                    vp = psA.tile([P, 1024], F32, tag="big", name="vp", bufs=3)
                    for i4 in range(4):
                        mc = qh * 4 + i4
                        nc.tensor.matmul(
                            vp[:, i4 * C:(i4 + 1) * C],
                            x8[:, 0:2, mc * P:(mc + 1) * P],
                            wvt8[:, 0:2, 0:C],
                            start=True, stop=True, perf_mode=DR)
                    nc.vector.tensor_copy(out=vT8[:, qh * 4:(qh + 1) * 4, :],
                                          in_=vp[:])
                ux_chain(2)
                ux_chain(3)

            # bias for the DVE bit-trick exp chunks
            nc.vector.tensor_scalar(uxb_col[:], ux_col[:], UXB_A, EXP_C,
                                    op0=MUL, op1=ADD)

            # ---- phase 2: transposed scores, exp (2 engines), denominator ----
            with tc.tile_pool(name="psS", bufs=2, space=bass.MemorySpace.PSUM) as psS:
                for mc in range(NMC):
                    s = psS.tile([P, L], F32, tag="s", name="s")
                    for ln in range(NLN):
                        col = ln * NB
                        nc.tensor.matmul(
                            s[:, col:col + NB],
                            w8[:, 0:2, mc * P:(mc + 1) * P],
                            x8[:, 0:2, col:col + NB],
                            start=True, stop=True, perf_mode=DR)
                    if mc in DVE_EXP_CHUNKS:
                        nc.vector.tensor_scalar(
                            pT8[:, mc, :].bitcast(I8), s[:],
                            EXP_A, uxb_col[:, mc, :], op0=MUL, op1=ADD)
                    else:
                        nc.scalar.activation(
                            out=pT8[:, mc, :],
                            in_=s[:], func=mybir.ActivationFunctionType.Exp,
                            scale=1.0 / 256.0, bias=ux_col[:, mc, :])
                    if mc == 0:
                        nc.vector.tensor_copy(out=dacc[:], in_=pT8[:, 0, :])
                    else:
                        nc.vector.tensor_add(dacc[:], dacc[:], pT8[:, mc, :])

            # ---- phase 3: context + denominator + epilogue ----
            with tc.tile_pool(name="psC", bufs=1, space=bass.MemorySpace.PSUM) as psC:
                ctx_t = {}

                def ctx_mms(qt):
                    cols = slice(qt * NB, (qt + 1) * NB)
                    for cc in range(2):
                        ct = psC.tile([P, NB], F32, tag="ctx", name="ctx", bufs=4)
                        ctx_t[(qt, cc)] = ct
                        for mp in range(NMC // 2):
                            nc.tensor.matmul(
                                ct[:],
                                vT8[:, 2 * mp:2 * mp + 2, cc * P:(cc + 1) * P],
                                pT8[:, 2 * mp:2 * mp + 2, cols],
                                start=(mp == 0), stop=(mp == NMC // 2 - 1),
                                perf_mode=DR)

                store_eng = [nc.sync, nc.scalar, nc.gpsimd]

                def ctx_evict(qt, nsub):
                    sub = NB // nsub
                    for cc in range(2):
                        rows = slice(cc * P, (cc + 1) * P)
                        for si in range(nsub):
                            c0 = qt * NB + si * sub
                            cols = slice(c0, c0 + sub)
                            pcols = slice(si * sub, (si + 1) * sub)
                            t = evict.tile([P, sub], F32, tag="t", name="t")
                            nc.vector.tensor_mul(t[:], ctx_t[(qt, cc)][:, pcols],
                                                 recip[:, cols])
                            o = evict.tile([P, sub], F32, tag="o", name="o")
                            nc.vector.scalar_tensor_tensor(
                                o[:], x_f[:, cc, cols], bv_sb[:, cc, :], t[:],
                                op0=ADD, op1=ADD)
                            eng = store_eng[(qt * 2 + cc + si) % 3]
                            eng.dma_start(out=out_d[rows, cols], in_=o[:])

                # quarter 0 accumulates while the denominator finishes on DVE
                ctx_mms(0)
                for ln in range(NLN):
                    cols = slice(ln * NB, (ln + 1) * NB)
                    ds = psC.tile([P, NB], F32, tag="d", name="d", bufs=2)
                    nc.tensor.matmul(ds[:], cst16[:, 0:P], dacc[:, cols],
                                     start=True, stop=True)
                    nc.vector.reciprocal_approx_fast(out=recip[:, cols],
                                                     in_=ds[:])
                for qt in range(1, NLN):
                    ctx_mms(qt)
                    ctx_evict(qt - 1, 1)
                ctx_evict(NLN - 1, 2)

    nc.compile()
    return nc


def get_compiled():
    global _COMPILED
    if _COMPILED is None:
        _COMPILED = build_nc()
    return _COMPILED


def make_in_maps(inputs):
    x = np.ascontiguousarray(np.asarray(inputs["x"], dtype=np.float32))
    Wq = np.asarray(inputs["Wq"], np.float32)
    Wk = np.asarray(inputs["Wk"], np.float32)
    Wv = np.asarray(inputs["Wv"], np.float32)
    M = Wq.T @ Wk                                   # scores = x^T M x
    u = SCALE * (Wk.T @ np.asarray(inputs["bq"], np.float32))
    u8 = np.zeros((C, 16), ml_dtypes.float8_e4m3)
    u8[:, 0] = (256.0 * u).astype(ml_dtypes.float8_e4m3)
    shared = {
        "mt8": np.ascontiguousarray((16.0 * M).T).astype(ml_dtypes.float8_e4m3),
        "wvt8": np.ascontiguousarray((16.0 * Wv).T).astype(ml_dtypes.float8_e4m3),
        "u8": u8,
        "bv": np.asarray(inputs["bv"], np.float32).reshape(C, 1),
    }
    return [{"x": x[i], "x8": x[i].astype(ml_dtypes.float8_e4m3), **shared}
            for i in range(B)]


def run(inputs, trace=False, **kwargs):
    nc = get_compiled()
    res = run_bass_kernel_spmd(nc, make_in_maps(inputs),
                               core_ids=list(range(B)), trace=trace, **kwargs)
    out = np.stack([res.results[i]["out"] for i in range(B)], axis=0)
    return out.astype(np.float32), res


def kernel(**inputs):
    out, _ = run(inputs)
    return out


# revision 3
# speedup vs baseline: 1.2127x; 1.2127x over previous
"""Trainium2 Bass kernel for AttentionBlock (B=8, C=256, L=2048), data-parallel
over batch across 8 NeuronCores.

Math (one batch per core, x: [C, L]):
    scores^T = x^T M x + (u.x) 1^T   with  M = Wq^T Wk,  u = Wk^T bq / sqrt(C)
    pT = exp(scores^T / sqrt(C) + ux)        [m, l], m on partitions
    denom = 16.ones^T pT   (PE DoubleRow matmuls over the fp8 pT tiles)
    ctx = vT^T pT,  vT = x^T Wv^T
    out = ctx * (1/denom16) + (x + bv)

All big matmuls run fp8e4m3 DoubleRow (2 contraction rows per PE cell; ~216ns
per 512-col matmul at steady clock = 2x bf16). Host scales M/Wv by 16 and u by
256 to keep fp8 operands in the normal range; the 16x on v cancels through the
denominator (its reduction matmul uses a constant-16 stationary tile) and the
256x on u is undone at the ux eviction.

exp splits across two engines: 10 chunks on ScalarE (ACTIVATE Exp, fp8 out),
6 on the DVE via a Schraudolph bit-trick - a single tensor_scalar computing
int8(scores*a + b) whose int8 bits ARE the fp8e4m3 encoding of exp.

All inputs ship pre-shuffled from the host as [128, ...] partition-major
arrays so every load is a contiguous per-partition DMA on the sync queue
(the earliest-clearing engine queue, ~2.5us).
"""

import numpy as np
import ml_dtypes

import concourse.bass as bass
import concourse.tile as tile
from concourse import bacc, mybir
from concourse.bass_utils import run_bass_kernel_spmd

B, C, L = 8, 256, 2048
P = 128                 # partitions
NMC = L // P            # 16 m-chunks (key blocks)
NB = 512                # matmul moving free dim (one PSUM bank)
NLN = L // NB           # 4 col slices of 512
SCALE = float(C) ** -0.5
WARMUP_MMS = 6

LN2 = float(np.log(2.0))
EXP_A = 8.0 / (256.0 * LN2)          # scores_psum -> fp8 bits slope
EXP_C = 56.0 + 0.5                   # fp8e4m3 exponent-bias magic + tweak
UXB_A = 8.0 / LN2                    # ux -> bits bias slope

DVE_EXP_CHUNKS = (2, 5, 7, 10, 12, 14)   # chunks whose exp runs on DVE

F32 = mybir.dt.float32
BF16 = mybir.dt.bfloat16
FP8 = mybir.dt.float8e4
I8 = mybir.dt.int8
DR = mybir.MatmulPerfMode.DoubleRow
MUL = mybir.AluOpType.mult
ADD = mybir.AluOpType.add

_COMPILED = None


def build_nc():
    nc = bacc.Bacc("TRN2", target_bir_lowering=False, debug=False, num_devices=8)

    # all inputs pre-shuffled host-side into partition-major layouts
    x_d = nc.dram_tensor("x", [P, 2, L], F32, kind="ExternalInput").ap()
    x8_d = nc.dram_tensor("x8", [P, 2, L], FP8, kind="ExternalInput").ap()
    mt8_d = nc.dram_tensor("mt8", [P, 2, C], FP8, kind="ExternalInput").ap()
    wvt8_d = nc.dram_tensor("wvt8", [P, 2, C], FP8, kind="ExternalInput").ap()
    u8_d = nc.dram_tensor("u8", [P, 2, 16], FP8, kind="ExternalInput").ap()
    bv_d = nc.dram_tensor("bv", [P, 2, 1], F32, kind="ExternalInput").ap()
    out_d = nc.dram_tensor("out", [C, L], F32, kind="ExternalOutput").ap()
    uxs_d = nc.dram_tensor("uxs", [1, L], F32).ap()      # scratch bounce

    with tile.TileContext(nc) as tc:
        with (
            tc.tile_pool(name="const", bufs=1) as const,
            tc.tile_pool(name="data", bufs=1) as data,
            tc.tile_pool(name="evict", bufs=4) as evict,
        ):
            # ---- constants ----
            c16 = const.tile([P, 2, NB], FP8)       # warmup + denominator lhsT
            nc.gpsimd.memset(c16[:], 16.0)

            x8 = data.tile([P, 2, L], FP8, tag="x8", name="x8")
            mt8 = const.tile([P, 2, C], FP8, tag="mt8")
            wvt8 = const.tile([P, 2, C], FP8, tag="wvt8")
            u8 = const.tile([P, 2, 16], FP8, tag="u8")
            bv_sb = const.tile([P, 2, 1], F32, tag="bv")
            x_f = data.tile([P, 2, L], F32, tag="xf", name="xf")

            # contiguous per-partition loads, all on the sync queue; weights
            # first (tiny + needed first), then the fp8 x, then fp32 x late
            nc.sync.dma_start(out=mt8[:], in_=mt8_d[:])
            nc.sync.dma_start(out=u8[:], in_=u8_d[:])
            nc.sync.dma_start(out=x8[:, :, 0:1024], in_=x8_d[:, :, 0:1024])
            nc.sync.dma_start(out=x8[:, :, 1024:2048], in_=x8_d[:, :, 1024:2048])
            nc.sync.dma_start(out=wvt8[:], in_=wvt8_d[:])
            nc.sync.dma_start(out=bv_sb[:], in_=bv_d[:])
            nc.gpsimd.dma_start(out=x_f[:], in_=x_d[:])

            w8 = data.tile([P, 2, L], FP8, tag="w8", name="w8")
            vT8 = data.tile([P, NMC, C], FP8, tag="vT8")
            pT8 = data.tile([P, NMC, L], FP8, tag="pT8")
            recip = data.tile([P, L], F32, tag="recip")
            ux_row = data.tile([1, L], F32, tag="uxrow")
            ux_col = data.tile([P, NMC, 1], F32, tag="uxcol")
            uxb_col = data.tile([P, NMC, 1], F32, tag="uxbcol")
            junk = data.tile([P, 16], BF16, tag="junk")

            # warm the exp activation table set while DMAs land
            nc.scalar.activation(out=junk[:], in_=c16[:, 0, 0:16],
                                 func=mybir.ActivationFunctionType.Exp)

            # ---- phase 1: projections ----
            with tc.tile_pool(name="psA", bufs=1, space=bass.MemorySpace.PSUM) as psA:
                warm = psA.tile([P, NB], F32, tag="warm", name="warm", bufs=1)
                for _ in range(WARMUP_MMS):
                    nc.tensor.matmul(warm[:], c16[:, 0:2, 0:P],
                                     c16[:, 0:2, 0:NB],
                                     start=True, stop=True, perf_mode=DR)

                def ux_chain(ln):
                    cols = slice(ln * NB, (ln + 1) * NB)
                    up = psA.tile([1, NB], F32, tag="up", name="up", bufs=1)
                    nc.tensor.matmul(up[0:1, :], u8[:, 0:2, 0:1],
                                     x8[:, 0:2, cols],
                                     start=True, stop=True, perf_mode=DR)
                    nc.vector.tensor_scalar_mul(ux_row[0:1, cols], up[:], 1.0 / 256.0)
                    nc.gpsimd.dma_start(out=uxs_d[0:1, cols], in_=ux_row[0:1, cols])
                    nc.gpsimd.dma_start(
                        out=ux_col[:, ln * 4:(ln + 1) * 4, :],
                        in_=uxs_d[0:1, cols].rearrange("o (a p) -> p a o", p=P))

                ux_chain(0)

                # w = (16M) x : DoubleRow contracts all 256 channels per mm
                for h in range(2):
                    for oc in range(2):
                        wp = psA.tile([P, 1024], F32, tag="big", name="wp", bufs=3)
                        for ln in range(2):
                            c0 = h * 1024 + ln * NB
                            nc.tensor.matmul(
                                wp[:, ln * NB:(ln + 1) * NB],
                                mt8[:, 0:2, oc * P:(oc + 1) * P],
                                x8[:, 0:2, c0:c0 + NB],
                                start=True, stop=True, perf_mode=DR)
                        nc.scalar.copy(out=w8[:, oc, h * 1024:(h + 1) * 1024],
                                       in_=wp[:])
                ux_chain(1)

                # vT[m, c] = sum_c' x[c', m] (16 WvT)[c', c]
                for qh in range(4):
                    vp = psA.tile([P, 1024], F32, tag="big", name="vp", bufs=3)
                    for i4 in range(4):
                        mc = qh * 4 + i4
                        nc.tensor.matmul(
                            vp[:, i4 * C:(i4 + 1) * C],
                            x8[:, 0:2, mc * P:(mc + 1) * P],
                            wvt8[:, 0:2, 0:C],
                            start=True, stop=True, perf_mode=DR)
                    nc.vector.tensor_copy(out=vT8[:, qh * 4:(qh + 1) * 4, :],
                                          in_=vp[:])
                ux_chain(2)
                ux_chain(3)

            # bias for the DVE bit-trick exp chunks
            nc.vector.tensor_scalar(uxb_col[:], ux_col[:], UXB_A, EXP_C,
                                    op0=MUL, op1=ADD)

            # ---- phase 2: transposed scores + exp on two engines ----
            with tc.tile_pool(name="psS", bufs=2, space=bass.MemorySpace.PSUM) as psS:
                for mc in range(NMC):
                    s = psS.tile([P, L], F32, tag="s", name="s")
                    for ln in range(NLN):
                        col = ln * NB
                        nc.tensor.matmul(
                            s[:, col:col + NB],
                            w8[:, 0:2, mc * P:(mc + 1) * P],
                            x8[:, 0:2, col:col + NB],
                            start=True, stop=True, perf_mode=DR)
                    if mc in DVE_EXP_CHUNKS:
                        nc.vector.tensor_scalar(
                            pT8[:, mc, :].bitcast(I8), s[:],
                            EXP_A, uxb_col[:, mc, :], op0=MUL, op1=ADD)
                    else:
                        nc.scalar.activation(
                            out=pT8[:, mc, :],
                            in_=s[:], func=mybir.ActivationFunctionType.Exp,
                            scale=1.0 / 256.0, bias=ux_col[:, mc, :])

            # ---- phase 3: context + PE denominator + epilogue ----
            with tc.tile_pool(name="psC", bufs=1, space=bass.MemorySpace.PSUM) as psC:
                ctx_t = {}
                ds = psC.tile([P, L], F32, tag="d", name="d", bufs=1)

                def ctx_mms(qt):
                    cols = slice(qt * NB, (qt + 1) * NB)
                    for cc in range(2):
                        ct = psC.tile([P, NB], F32, tag="ctx", name="ctx", bufs=4)
                        ctx_t[(qt, cc)] = ct
                        for mp in range(NMC // 2):
                            nc.tensor.matmul(
                                ct[:],
                                vT8[:, 2 * mp:2 * mp + 2, cc * P:(cc + 1) * P],
                                pT8[:, 2 * mp:2 * mp + 2, cols],
                                start=(mp == 0), stop=(mp == NMC // 2 - 1),
                                perf_mode=DR)

                def den_mms(ln):
                    cols = slice(ln * NB, (ln + 1) * NB)
                    for mp in range(NMC // 2):
                        nc.tensor.matmul(
                            ds[:, cols], c16[:, 0:2, 0:P],
                            pT8[:, 2 * mp:2 * mp + 2, cols],
                            start=(mp == 0), stop=(mp == NMC // 2 - 1),
                            perf_mode=DR)
                    nc.vector.reciprocal_approx_fast(out=recip[:, cols],
                                                     in_=ds[:, cols])

                store_eng = [nc.sync, nc.scalar, nc.gpsimd]

                def ctx_evict(qt, nsub):
                    sub = NB // nsub
                    for cc in range(2):
                        rows = slice(cc * P, (cc + 1) * P)
                        for si in range(nsub):
                            c0 = qt * NB + si * sub
                            cols = slice(c0, c0 + sub)
                            pcols = slice(si * sub, (si + 1) * sub)
                            t = evict.tile([P, sub], F32, tag="t", name="t")
                            nc.vector.tensor_mul(t[:], ctx_t[(qt, cc)][:, pcols],
                                                 recip[:, cols])
                            o = evict.tile([P, sub], F32, tag="o", name="o")
                            nc.vector.scalar_tensor_tensor(
                                o[:], x_f[:, cc, cols], bv_sb[:, cc, :], t[:],
                                op0=ADD, op1=ADD)
                            eng = store_eng[(qt * 2 + cc + si) % 3]
                            eng.dma_start(out=out_d[rows, cols], in_=o[:])

                den_mms(0)
                ctx_mms(0)
                den_mms(1)
                ctx_mms(1)
                den_mms(2)
                ctx_evict(0, 1)
                ctx_mms(2)
                den_mms(3)
                ctx_evict(1, 1)
                ctx_mms(3)
                ctx_evict(2, 1)
                ctx_evict(3, 2)

    nc.compile()
    return nc


def get_compiled():
    global _COMPILED
    if _COMPILED is None:
        _COMPILED = build_nc()
    return _COMPILED


def _shuffle(a):
    """[2*P, N...] -> [P, 2, N...] partition-major."""
    return np.ascontiguousarray(a.reshape(2, P, *a.shape[1:]).transpose(1, 0, *range(2, a.ndim + 1)))


def make_in_maps(inputs):
    x = np.ascontiguousarray(np.asarray(inputs["x"], dtype=np.float32))
    Wq = np.asarray(inputs["Wq"], np.float32)
    Wk = np.asarray(inputs["Wk"], np.float32)
    Wv = np.asarray(inputs["Wv"], np.float32)
    M = Wq.T @ Wk                                   # scores = x^T M x
    u = SCALE * (Wk.T @ np.asarray(inputs["bq"], np.float32))
    u8 = np.zeros((C, 16), ml_dtypes.float8_e4m3)
    u8[:, 0] = (256.0 * u).astype(ml_dtypes.float8_e4m3)
    shared = {
        "mt8": _shuffle(np.ascontiguousarray((16.0 * M).T).astype(ml_dtypes.float8_e4m3)),
        "wvt8": _shuffle(np.ascontiguousarray((16.0 * Wv).T).astype(ml_dtypes.float8_e4m3)),
        "u8": _shuffle(u8),
        "bv": _shuffle(np.asarray(inputs["bv"], np.float32).reshape(C, 1)),
    }
    return [{"x": _shuffle(x[i]), "x8": _shuffle(x[i].astype(ml_dtypes.float8_e4m3)),
             **shared} for i in range(B)]


def run(inputs, trace=False, **kwargs):
    nc = get_compiled()
    res = run_bass_kernel_spmd(nc, make_in_maps(inputs),
                               core_ids=list(range(B)), trace=trace, **kwargs)
    out = np.stack([res.results[i]["out"] for i in range(B)], axis=0)
    return out.astype(np.float32), res


def kernel(**inputs):
    out, _ = run(inputs)
    return out


# revision 4
# speedup vs baseline: 1.2202x; 1.0062x over previous
"""Trainium2 Bass kernel for AttentionBlock (B=8, C=256, L=2048), data-parallel
over batch across 8 NeuronCores.

Math (one batch per core, x: [C, L]):
    scores^T = x^T M x + (u.x) 1^T   with  M = Wq^T Wk,  u = Wk^T bq / sqrt(C)
    pT = exp(scores^T / sqrt(C) + ux)        [m, l], m on partitions
    denom = 16.ones^T pT   (PE DoubleRow matmuls over the fp8 pT tiles)
    ctx = vT^T pT,  vT = x^T Wv^T
    out = ctx * (1/denom16) + (x + bv)

All big matmuls run fp8e4m3 DoubleRow (2 contraction rows per PE cell; ~216ns
per 512-col matmul at steady clock = 2x bf16). Host scales M/Wv by 16 and u by
256 to keep fp8 operands in the normal range; the 16x on v cancels through the
denominator (constant-16 stationary on its reduction matmuls) and the 256x on
u is undone at the ux eviction.

Every [128,2048] exp chunk is split across BOTH post-PE engines concurrently:
ScalarE runs ACTIVATE Exp on cols [0:1120] while the DVE computes cols
[1120:2048] via a Schraudolph bit-trick (one tensor_scalar emitting int8
(scores*a + b) whose int8 bits ARE fp8e4m3 exp). Splitting per chunk keeps
the PSUM ring latency at ~1.2us/chunk instead of serializing whole-chunk
exps behind the score matmul bursts.

Phase 3 runs per 512-col quarter: denominator matmuls -> reciprocal -> context
matmuls -> eviction (ctx*recip fused with the bf16 residual + bv in two DVE
ops) -> store, so output DMA spreads across the whole phase.
"""

import numpy as np
import ml_dtypes

import concourse.bass as bass
import concourse.tile as tile
from concourse import bacc, mybir
from concourse.bass_utils import run_bass_kernel_spmd

B, C, L = 8, 256, 2048
P = 128                 # partitions
NMC = L // P            # 16 m-chunks (key blocks)
NB = 512                # matmul moving free dim (one PSUM bank)
NLN = L // NB           # 4 col slices of 512
SCALE = float(C) ** -0.5
WARMUP_MMS = 5
SSPLIT = 1120           # exp cols on ScalarE; rest on DVE

LN2 = float(np.log(2.0))
EXP_A = 8.0 / (256.0 * LN2)          # scores_psum -> fp8 bits slope
EXP_C = 56.0 + 0.5                   # fp8e4m3 exponent-bias magic + tweak
UXB_A = 8.0 / LN2                    # ux -> bits bias slope

F32 = mybir.dt.float32
BF16 = mybir.dt.bfloat16
FP8 = mybir.dt.float8e4
I8 = mybir.dt.int8
DR = mybir.MatmulPerfMode.DoubleRow
MUL = mybir.AluOpType.mult
ADD = mybir.AluOpType.add

_COMPILED = None


def build_nc():
    nc = bacc.Bacc("TRN2", target_bir_lowering=False, debug=False, num_devices=8)

    # all inputs pre-shuffled host-side into partition-major layouts
    xbf_d = nc.dram_tensor("xbf", [P, 2, L], BF16, kind="ExternalInput").ap()
    x8_d = nc.dram_tensor("x8", [P, 2, L], FP8, kind="ExternalInput").ap()
    mt8_d = nc.dram_tensor("mt8", [P, 2, C], FP8, kind="ExternalInput").ap()
    wvt8_d = nc.dram_tensor("wvt8", [P, 2, C], FP8, kind="ExternalInput").ap()
    u8_d = nc.dram_tensor("u8", [P, 2, 16], FP8, kind="ExternalInput").ap()
    bv_d = nc.dram_tensor("bv", [P, 2, 1], F32, kind="ExternalInput").ap()
    out_d = nc.dram_tensor("out", [C, L], F32, kind="ExternalOutput").ap()
    uxs_d = nc.dram_tensor("uxs", [1, L], F32).ap()      # scratch bounce

    with tile.TileContext(nc) as tc:
        with (
            tc.tile_pool(name="const", bufs=1) as const,
            tc.tile_pool(name="data", bufs=1) as data,
            tc.tile_pool(name="evict", bufs=4) as evict,
        ):
            # ---- constants ----
            c16 = const.tile([P, 2, NB], FP8)       # warmup + denominator lhsT
            nc.gpsimd.memset(c16[:], 16.0)

            x8 = data.tile([P, 2, L], FP8, tag="x8", name="x8")
            mt8 = const.tile([P, 2, C], FP8, tag="mt8")
            wvt8 = const.tile([P, 2, C], FP8, tag="wvt8")
            u8 = const.tile([P, 2, 16], FP8, tag="u8")
            bv_sb = const.tile([P, 2, 1], F32, tag="bv")
            x_bf = data.tile([P, 2, L], BF16, tag="xbf", name="xbf")

            # critical loads on the sync queue, weights first
            nc.sync.dma_start(out=mt8[:], in_=mt8_d[:])
            nc.sync.dma_start(out=u8[:], in_=u8_d[:])
            nc.sync.dma_start(out=x8[:, :, 0:1024], in_=x8_d[:, :, 0:1024])
            nc.sync.dma_start(out=x8[:, :, 1024:2048], in_=x8_d[:, :, 1024:2048])
            nc.sync.dma_start(out=wvt8[:], in_=wvt8_d[:])
            nc.sync.dma_start(out=bv_sb[:], in_=bv_d[:])
            # bf16 residual: only needed by the phase-3 epilogue
            nc.sync.dma_start(out=x_bf[:, 0, :], in_=xbf_d[:, 0, :])

            w8 = data.tile([P, 2, L], FP8, tag="w8", name="w8")
            vT8 = data.tile([P, NMC, C], FP8, tag="vT8")
            pT8 = data.tile([P, NMC, L], FP8, tag="pT8")
            recip = data.tile([P, L], F32, tag="recip")
            ux_row = data.tile([1, L], F32, tag="uxrow")
            ux_col = data.tile([P, NMC, 1], F32, tag="uxcol")
            uxb_col = data.tile([P, NMC, 1], F32, tag="uxbcol")
            junk = data.tile([P, 16], BF16, tag="junk")

            # warm the exp activation table set while DMAs land; second half
            # of the residual rides the scalar queue behind it
            nc.scalar.activation(out=junk[:], in_=c16[:, 0, 0:16],
                                 func=mybir.ActivationFunctionType.Exp)
            nc.scalar.dma_start(out=x_bf[:, 1, :], in_=xbf_d[:, 1, :])

            # ---- phase 1: projections ----
            with tc.tile_pool(name="psA", bufs=1, space=bass.MemorySpace.PSUM) as psA:
                warm = psA.tile([P, NB], F32, tag="warm", name="warm", bufs=1)
                for _ in range(WARMUP_MMS):
                    nc.tensor.matmul(warm[:], c16[:, 0:2, 0:P],
                                     c16[:, 0:2, 0:NB],
                                     start=True, stop=True, perf_mode=DR)

                def ux_chain(ln):
                    cols = slice(ln * NB, (ln + 1) * NB)
                    up = psA.tile([1, NB], F32, tag="up", name="up", bufs=1)
                    nc.tensor.matmul(up[0:1, :], u8[:, 0:2, 0:1],
                                     x8[:, 0:2, cols],
                                     start=True, stop=True, perf_mode=DR)
                    nc.vector.tensor_scalar_mul(ux_row[0:1, cols], up[:], 1.0 / 256.0)
                    nc.gpsimd.dma_start(out=uxs_d[0:1, cols], in_=ux_row[0:1, cols])
                    nc.gpsimd.dma_start(
                        out=ux_col[:, ln * 4:(ln + 1) * 4, :],
                        in_=uxs_d[0:1, cols].rearrange("o (a p) -> p a o", p=P))

                ux_chain(0)

                # w = (16M) x : DoubleRow contracts all 256 channels per mm
                for h in range(2):
                    for oc in range(2):
                        wp = psA.tile([P, 1024], F32, tag="big", name="wp", bufs=3)
                        for ln in range(2):
                            c0 = h * 1024 + ln * NB
                            nc.tensor.matmul(
                                wp[:, ln * NB:(ln + 1) * NB],
                                mt8[:, 0:2, oc * P:(oc + 1) * P],
                                x8[:, 0:2, c0:c0 + NB],
                                start=True, stop=True, perf_mode=DR)
                        nc.scalar.copy(out=w8[:, oc, h * 1024:(h + 1) * 1024],
                                       in_=wp[:])
                ux_chain(1)

                # vT[m, c] = sum_c' x[c', m] (16 WvT)[c', c]
                for qh in range(4):
                    vp = psA.tile([P, 1024], F32, tag="big", name="vp", bufs=3)
                    for i4 in range(4):
                        mc = qh * 4 + i4
                        nc.tensor.matmul(
                            vp[:, i4 * C:(i4 + 1) * C],
                            x8[:, 0:2, mc * P:(mc + 1) * P],
                            wvt8[:, 0:2, 0:C],
                            start=True, stop=True, perf_mode=DR)
                    nc.vector.tensor_copy(out=vT8[:, qh * 4:(qh + 1) * 4, :],
                                          in_=vp[:])
                ux_chain(2)
                ux_chain(3)

            # bias for the DVE bit-trick exp slices
            nc.vector.tensor_scalar(uxb_col[:], ux_col[:], UXB_A, EXP_C,
                                    op0=MUL, op1=ADD)

            # ---- phase 2: transposed scores + split exp on both engines ----
            with tc.tile_pool(name="psS", bufs=2, space=bass.MemorySpace.PSUM) as psS:
                for mc in range(NMC):
                    s = psS.tile([P, L], F32, tag="s", name="s")
                    for ln in range(NLN):
                        col = ln * NB
                        nc.tensor.matmul(
                            s[:, col:col + NB],
                            w8[:, 0:2, mc * P:(mc + 1) * P],
                            x8[:, 0:2, col:col + NB],
                            start=True, stop=True, perf_mode=DR)
                    nc.scalar.activation(
                        out=pT8[:, mc, 0:SSPLIT],
                        in_=s[:, 0:SSPLIT],
                        func=mybir.ActivationFunctionType.Exp,
                        scale=1.0 / 256.0, bias=ux_col[:, mc, :])
                    nc.vector.tensor_scalar(
                        pT8[:, mc, SSPLIT:L].bitcast(I8), s[:, SSPLIT:L],
                        EXP_A, uxb_col[:, mc, :], op0=MUL, op1=ADD)

            # ---- phase 3: per-quarter denominator + context + epilogue ----
            with tc.tile_pool(name="psC", bufs=1, space=bass.MemorySpace.PSUM) as psC:
                ds = psC.tile([P, L], F32, tag="d", name="d", bufs=1)
                ctx_t = {}
                store_eng = [nc.sync, nc.scalar, nc.gpsimd]

                def den_mms(qt):
                    cols = slice(qt * NB, (qt + 1) * NB)
                    for mp in range(NMC // 2):
                        nc.tensor.matmul(
                            ds[:, cols], c16[:, 0:2, 0:P],
                            pT8[:, 2 * mp:2 * mp + 2, cols],
                            start=(mp == 0), stop=(mp == NMC // 2 - 1),
                            perf_mode=DR)
                    nc.vector.reciprocal_approx_fast(out=recip[:, cols],
                                                     in_=ds[:, cols])

                def ctx_mms(qt):
                    cols = slice(qt * NB, (qt + 1) * NB)
                    for cc in range(2):
                        ct = psC.tile([P, NB], F32, tag="ctx", name="ctx", bufs=4)
                        ctx_t[(qt, cc)] = ct
                        for mp in range(NMC // 2):
                            nc.tensor.matmul(
                                ct[:],
                                vT8[:, 2 * mp:2 * mp + 2, cc * P:(cc + 1) * P],
                                pT8[:, 2 * mp:2 * mp + 2, cols],
                                start=(mp == 0), stop=(mp == NMC // 2 - 1),
                                perf_mode=DR)

                def ctx_evict(qt, nsub):
                    sub = NB // nsub
                    for cc in range(2):
                        rows = slice(cc * P, (cc + 1) * P)
                        for si in range(nsub):
                            c0 = qt * NB + si * sub
                            cols = slice(c0, c0 + sub)
                            pcols = slice(si * sub, (si + 1) * sub)
                            t = evict.tile([P, sub], F32, tag="t", name="t")
                            nc.vector.tensor_mul(t[:], ctx_t[(qt, cc)][:, pcols],
                                                 recip[:, cols])
                            o = evict.tile([P, sub], F32, tag="o", name="o")
                            nc.vector.scalar_tensor_tensor(
                                o[:], x_bf[:, cc, cols], bv_sb[:, cc, :], t[:],
                                op0=ADD, op1=ADD)
                            eng = store_eng[(qt * 2 + cc + si) % 3]
                            eng.dma_start(out=out_d[rows, cols], in_=o[:])

                for qt in range(NLN):
                    den_mms(qt)
                    ctx_mms(qt)
                    ctx_evict(qt, 1 if qt < NLN - 1 else 2)

    nc.compile()
    return nc


def get_compiled():
    global _COMPILED
    if _COMPILED is None:
        _COMPILED = build_nc()
    return _COMPILED


def _shuffle(a):
    """[2*P, N...] -> [P, 2, N...] partition-major."""
    return np.ascontiguousarray(a.reshape(2, P, *a.shape[1:]).transpose(1, 0, *range(2, a.ndim + 1)))


def make_in_maps(inputs):
    x = np.ascontiguousarray(np.asarray(inputs["x"], dtype=np.float32))
    Wq = np.asarray(inputs["Wq"], np.float32)
    Wk = np.asarray(inputs["Wk"], np.float32)
    Wv = np.asarray(inputs["Wv"], np.float32)
    M = Wq.T @ Wk                                   # scores = x^T M x
    u = SCALE * (Wk.T @ np.asarray(inputs["bq"], np.float32))
    u8 = np.zeros((C, 16), ml_dtypes.float8_e4m3)
    u8[:, 0] = (256.0 * u).astype(ml_dtypes.float8_e4m3)
    shared = {
        "mt8": _shuffle(np.ascontiguousarray((16.0 * M).T).astype(ml_dtypes.float8_e4m3)),
        "wvt8": _shuffle(np.ascontiguousarray((16.0 * Wv).T).astype(ml_dtypes.float8_e4m3)),
        "u8": _shuffle(u8),
        "bv": _shuffle(np.asarray(inputs["bv"], np.float32).reshape(C, 1)),
    }
    return [{"xbf": _shuffle(x[i].astype(ml_dtypes.bfloat16)),
             "x8": _shuffle(x[i].astype(ml_dtypes.float8_e4m3)),
             **shared} for i in range(B)]


def run(inputs, trace=False, **kwargs):
    nc = get_compiled()
    res = run_bass_kernel_spmd(nc, make_in_maps(inputs),
                               core_ids=list(range(B)), trace=trace, **kwargs)
    out = np.stack([res.results[i]["out"] for i in range(B)], axis=0)
    return out.astype(np.float32), res


def kernel(**inputs):
    out, _ = run(inputs)
    return out
